# revision 4
# baseline (speedup 1.0000x reference)
"""Bass/Trainium2 kernel for nn_EntangleComplex.

The reference computes (x_real @ op, x_imag @ op) where op is a DIAGONAL
matrix with +-1 entries (elementwise product of diagonal CZ-style gates).
Hence x @ op == x * diag(op)[None, :] exactly (IEEE: off-diagonal terms
are exact zeros).  The device kernel is therefore a DMA-bound elementwise
multiply by a broadcast sign vector, data-parallel over the batch dim
across 8 NeuronCores with no communication.
"""

import numpy as np

import concourse.bacc as bacc
import concourse.mybir as mybir
import concourse.tile as tile
from concourse.bass_utils import run_bass_kernel_spmd

N_CORES = 8
BATCH = 4096
DIM = 4096
ROWS = BATCH // N_CORES  # 512 rows of each of x_real/x_imag per core
P = 128                  # SBUF partition count

_NC = None


def _build_program():
    global _NC
    if _NC is not None:
        return _NC
    nc = bacc.Bacc()
    dt = mybir.dt.float32
    xr = nc.declare_dram_parameter("xr", [ROWS, DIM], dt, isOutput=False)
    xi = nc.declare_dram_parameter("xi", [ROWS, DIM], dt, isOutput=False)
    d = nc.declare_dram_parameter("d", [P, DIM], dt, isOutput=False)
    yr = nc.declare_dram_parameter("yr", [ROWS, DIM], dt, isOutput=True)
    yi = nc.declare_dram_parameter("yi", [ROWS, DIM], dt, isOutput=True)

    with tile.TileContext(nc) as tc:
        with (
            tc.tile_pool(name="dpool", bufs=1) as dpool,
            tc.tile_pool(name="xpool", bufs=6) as xpool,
        ):
            dtile = dpool.tile([P, DIM], dt)
            nc.sync.dma_start(dtile[:], d[:])
            for src, dst in ((xr, yr), (xi, yi)):
                for i in range(ROWS // P):
                    t = xpool.tile([P, DIM], dt, tag="x")
                    nc.sync.dma_start(t[:], src[i * P:(i + 1) * P, :])
                    nc.vector.tensor_mul(t[:], t[:], dtile[:])
                    nc.sync.dma_start(dst[i * P:(i + 1) * P, :], t[:])
    nc.finalize()
    _NC = nc
    return nc


def kernel(x_real, x_imag, op):
    x_real = np.ascontiguousarray(np.asarray(x_real, dtype=np.float32))
    x_imag = np.ascontiguousarray(np.asarray(x_imag, dtype=np.float32))
    op = np.asarray(op, dtype=np.float32)
    dvec = np.ascontiguousarray(np.diagonal(op))
    db = np.ascontiguousarray(np.broadcast_to(dvec[None, :], (P, DIM)))

    nc = _build_program()
    in_maps = []
    for c in range(N_CORES):
        sl = slice(c * ROWS, (c + 1) * ROWS)
        in_maps.append({"xr": x_real[sl], "xi": x_imag[sl], "d": db})
    res = run_bass_kernel_spmd(nc, in_maps, list(range(N_CORES))).results
    y_real = np.concatenate([r["yr"] for r in res], axis=0)
    y_imag = np.concatenate([r["yi"] for r in res], axis=0)
    return y_real, y_imag


# revision 5
# speedup vs baseline: 1.0044x; 1.0044x over previous
"""Bass/Trainium2 kernel for nn_EntangleComplex.

The reference computes (x_real @ op, x_imag @ op) where op is a DIAGONAL
matrix with +-1 entries (elementwise product of diagonal CZ-style gates).
Hence x @ op == x * diag(op)[None, :] exactly (IEEE: off-diagonal terms
are exact zeros).  The device kernel is therefore a DMA-bound elementwise
multiply by a broadcast sign vector, data-parallel over the batch dim
across 8 NeuronCores with no communication.

Per core: 512 rows of x_real + 512 rows of x_imag (16 MiB in, 16 MiB
out).  The sign vector is DMA'd as one 16 KiB row and broadcast to all
128 SBUF partitions with a K=1 PE matmul against a ones vector, so DMA
traffic stays at the 32 MiB roofline.
"""

import numpy as np

import concourse.bacc as bacc
import concourse.mybir as mybir
import concourse.tile as tile
from concourse.bass_utils import run_bass_kernel_spmd

N_CORES = 8
BATCH = 4096
DIM = 4096
ROWS = BATCH // N_CORES  # 512 rows of each of x_real/x_imag per core
P = 128                  # SBUF partition count
MM_N = 512               # PSUM bank free-dim limit per matmul

_NC = None


def _build_program():
    global _NC
    if _NC is not None:
        return _NC
    nc = bacc.Bacc()
    dt = mybir.dt.float32
    xr = nc.declare_dram_parameter("xr", [ROWS, DIM], dt, isOutput=False)
    xi = nc.declare_dram_parameter("xi", [ROWS, DIM], dt, isOutput=False)
    d = nc.declare_dram_parameter("d", [1, DIM], dt, isOutput=False)
    yr = nc.declare_dram_parameter("yr", [ROWS, DIM], dt, isOutput=True)
    yi = nc.declare_dram_parameter("yi", [ROWS, DIM], dt, isOutput=True)

    with tile.TileContext(nc) as tc:
        with (
            tc.tile_pool(name="const", bufs=1) as cpool,
            tc.tile_pool(name="psum", bufs=2, space="PSUM") as ppool,
            tc.tile_pool(name="xpool", bufs=8) as xpool,
        ):
            dsmall = cpool.tile([1, DIM], dt)
            ones = cpool.tile([1, P], dt)
            dtile = cpool.tile([P, DIM], dt)
            nc.sync.dma_start(dsmall[:], d[:])
            nc.vector.memset(ones[:], 1.0)
            for j in range(DIM // MM_N):
                pt = ppool.tile([P, MM_N], dt)
                nc.tensor.matmul(
                    pt[:], ones[:], dsmall[0:1, j * MM_N:(j + 1) * MM_N]
                )
                nc.vector.tensor_copy(dtile[:, j * MM_N:(j + 1) * MM_N], pt[:])
            for src, dst in ((xr, yr), (xi, yi)):
                for i in range(ROWS // P):
                    t = xpool.tile([P, DIM], dt, tag="x")
                    nc.sync.dma_start(t[:], src[i * P:(i + 1) * P, :])
                    nc.vector.tensor_mul(t[:], t[:], dtile[:])
                    nc.sync.dma_start(dst[i * P:(i + 1) * P, :], t[:])
    nc.finalize()
    _NC = nc
    return nc


def kernel(x_real, x_imag, op):
    x_real = np.ascontiguousarray(np.asarray(x_real, dtype=np.float32))
    x_imag = np.ascontiguousarray(np.asarray(x_imag, dtype=np.float32))
    op = np.asarray(op, dtype=np.float32)
    dvec = np.ascontiguousarray(np.diagonal(op)).reshape(1, DIM)

    nc = _build_program()
    in_maps = []
    for c in range(N_CORES):
        sl = slice(c * ROWS, (c + 1) * ROWS)
        in_maps.append({"xr": x_real[sl], "xi": x_imag[sl], "d": dvec})
    res = run_bass_kernel_spmd(nc, in_maps, list(range(N_CORES))).results
    y_real = np.concatenate([r["yr"] for r in res], axis=0)
    y_imag = np.concatenate([r["yi"] for r in res], axis=0)
    return y_real, y_imag


# revision 6
# speedup vs baseline: 1.2694x; 1.2637x over previous
"""Bass/Trainium2 kernel for nn_EntangleComplex.

The reference computes (x_real @ op, x_imag @ op) where op is a DIAGONAL
matrix with +-1 entries (elementwise product of diagonal CZ-style gates).
Hence x @ op == x * diag(op)[None, :] exactly (IEEE: off-diagonal terms
are exact zeros).  The device kernel is therefore a DMA-bound elementwise
multiply by a broadcast sign vector, data-parallel over the batch dim
across 8 NeuronCores with no communication.

Per core: 512 rows of x_real + 512 rows of x_imag (16 MiB in, 16 MiB
out).  The sign vector is DMA'd as one 16 KiB row and broadcast to all
128 SBUF partitions with a K=1 PE matmul against a ones vector, so DMA
traffic stays at the 32 MiB roofline.
"""

import numpy as np

import concourse.bacc as bacc
import concourse.mybir as mybir
import concourse.tile as tile
from concourse.bass_utils import run_bass_kernel_spmd

N_CORES = 8
BATCH = 4096
DIM = 4096
ROWS = BATCH // N_CORES  # 512 rows of each of x_real/x_imag per core
P = 128                  # SBUF partition count
MM_N = 512               # PSUM bank free-dim limit per matmul

_NC = None


def _build_program():
    global _NC
    if _NC is not None:
        return _NC
    nc = bacc.Bacc()
    dt = mybir.dt.float32
    xr = nc.declare_dram_parameter("xr", [ROWS, DIM], dt, isOutput=False)
    xi = nc.declare_dram_parameter("xi", [ROWS, DIM], dt, isOutput=False)
    d = nc.declare_dram_parameter("d", [1, DIM], dt, isOutput=False)
    yr = nc.declare_dram_parameter("yr", [ROWS, DIM], dt, isOutput=True)
    yi = nc.declare_dram_parameter("yi", [ROWS, DIM], dt, isOutput=True)

    with tile.TileContext(nc) as tc:
        with (
            tc.tile_pool(name="const", bufs=1) as cpool,
            tc.tile_pool(name="psum", bufs=2, space="PSUM") as ppool,
            tc.tile_pool(name="xpool", bufs=8) as xpool,
        ):
            dsmall = cpool.tile([1, DIM], dt)
            ones = cpool.tile([1, P], dt)
            dtile = cpool.tile([P, DIM], dt)
            nc.sync.dma_start(dsmall[:], d[:])
            nc.vector.memset(ones[:], 1.0)
            for j in range(DIM // MM_N):
                pt = ppool.tile([P, MM_N], dt)
                nc.tensor.matmul(
                    pt[:], ones[:], dsmall[0:1, j * MM_N:(j + 1) * MM_N]
                )
                nc.vector.tensor_copy(dtile[:, j * MM_N:(j + 1) * MM_N], pt[:])
            # loads on SP sequencer, stores on Activation sequencer: a
            # store's semaphore wait must never block later load issues
            for src, dst in ((xr, yr), (xi, yi)):
                for i in range(ROWS // P):
                    t = xpool.tile([P, DIM], dt, tag="x")
                    nc.sync.dma_start(t[:], src[i * P:(i + 1) * P, :])
                    nc.vector.tensor_mul(t[:], t[:], dtile[:])
                    nc.scalar.dma_start(dst[i * P:(i + 1) * P, :], t[:])
    nc.finalize()
    _NC = nc
    return nc


def kernel(x_real, x_imag, op):
    x_real = np.ascontiguousarray(np.asarray(x_real, dtype=np.float32))
    x_imag = np.ascontiguousarray(np.asarray(x_imag, dtype=np.float32))
    op = np.asarray(op, dtype=np.float32)
    dvec = np.ascontiguousarray(np.diagonal(op)).reshape(1, DIM)

    nc = _build_program()
    in_maps = []
    for c in range(N_CORES):
        sl = slice(c * ROWS, (c + 1) * ROWS)
        in_maps.append({"xr": x_real[sl], "xi": x_imag[sl], "d": dvec})
    res = run_bass_kernel_spmd(nc, in_maps, list(range(N_CORES))).results
    y_real = np.concatenate([r["yr"] for r in res], axis=0)
    y_imag = np.concatenate([r["yi"] for r in res], axis=0)
    return y_real, y_imag
